# revision 56
# baseline (speedup 1.0000x reference)
"""Multi-head self-attention (B=2, T=2048, D=1024, 16 heads) on 8 TRN2 cores.

Sharding: core c = (b, g) with b = c // 4 (batch), g = c % 4 (head group of 4).
Each core computes q/k/v projections for its 4 heads, causal softmax
attention, and a partial output projection (its 256 columns of the
concat-head dim against Wo). Host sums the 4 partials per batch and adds bo.

Single merged pipeline, ordered so the PE never idles (keeps the HAM clock
gate warm): projection "granules" (one 512-token key block: kT, vt, qT) are
interleaved chunk-by-chunk with the attention stream of the previous key
block and with output-projection blocks of the block before that.

  granule n:  kT/qT [256, 512-slice] (transposed projections, head pairs
              stacked on partitions) and vt [512, 260] natural (per head 64
              value cols + a ones col that makes the AV matmul emit softmax
              denominators).
  B(J, hp):   per key-chunk kc: scoresT [128, 2x512] for both heads via
              tile_position row packing; exp on the scalar ACT (Exp LUT,
              scale=8, bias=ln K) or the vector engine (custom EXP8M op:
              monic cubic + 3 squarings, K-scaled) -- both emit
              K*exp(0.125 r) so K cancels in the softmax; causal masks via
              gpsimd affine_select; AV accumulates [v|1].T @ ex into
              at [65, 512] (row 64 = denominators). Normalize: denominators
              staged by scalar copy, reciprocal_approx_fast on DVE, gpsimd
              partition broadcast, DVE multiply into attT -- staged across
              the next block so no in-order engine queue ever stalls.
  C(t):       O[t-block] = attT.T @ WoS accumulated over head pairs in PSUM,
              copied to bf16 and DMA'd out; host converts/sums partials.
"""

import ml_dtypes
import numpy as np

import concourse.bass as bass
import concourse.tile as tile
from concourse import bacc, mybir
from concourse import bass_utils
from concourse import dve_ops
from concourse.dve_ops import DveOp
from concourse.dve_spec import Spec, Src0, C0, C1, C2, sq, lower as dve_lower
from concourse.dve_uop import DveOpSpec
from contextlib import ExitStack

F32 = mybir.dt.float32
F32R = mybir.dt.float32r
BF16 = mybir.dt.bfloat16
ATT = BF16  # dtype for attention-phase matmul operands
AF = mybir.ActivationFunctionType
OP = mybir.AluOpType

B, T, D = 2, 2048, 1024
NH, DH = 16, 64
HPC = 4            # heads per core
GD = HPC * DH      # 256, group dim
GV = HPC * (DH + 1)  # 260, v tile width
NKD = D // 128     # 8 K-chunks for projections
NT = T // 128      # 16 token chunks
NJ = T // 512      # 4 query blocks

_NC_CACHE = {}

# exp via monic cubic + 3 squarings on the vector engine:
#   p(u) = ((u + A2) u + A1) u + A0;  p(u)^8 ~= EXPK * exp(8u)  on |u| <= 0.47
# (max rel err 2.2e-4 in fp32). Scores arrive in PSUM pre-scaled to u = r/64
# (0.125/8 folded into Wq host-side); the ACT path uses scale=8, bias=ln(EXPK)
# so both engines emit EXPK * exp(0.125 r) and EXPK cancels in the softmax.
EXP_A2 = 3.06702906
EXP_A1 = 6.02255865
EXP_A0 = 6.01835402
EXP_LNK = 14.36056232


def _register_exp8m():
    if "EXP8M" in dve_ops._SUB_OPCODE_FOR_NAME:
        return next(o for o in dve_ops.OPS if o.name == "EXP8M")
    _p = ((Src0 + C0) * Src0 + C1) * Src0 + C2
    _body = sq(sq(sq(_p)))

    def _ref(in0, in1, s0, s1, imm2):
        p = (((in0 + s0) * in0 + s1) * in0 + imm2).astype(np.float32)
        return ((p * p) ** 2) ** 2

    spec = Spec(body=_body, reference=_ref)
    ver = "v3"
    sha = DveOpSpec(name="EXP8M", opcode=1, uops=dve_lower(spec, ver=ver),
                    rd1_en=False).sha(ver)
    op = DveOp("EXP8M", spec, subdim=False, uops_sha={ver: sha})
    dve_ops.OPS.append(op)
    dve_ops.CUSTOM_DVE_SPECS[op.name] = op.spec
    dve_ops._SUB_OPCODE_FOR_NAME[op.name] = (
        dve_ops._CUSTOM_DVE_ROW_BASE + len(dve_ops.OPS) - 1
    )
    return op


def build():
    if "nc" in _NC_CACHE:
        return _NC_CACHE["nc"]
    exp8m = _register_exp8m()
    nc = bacc.Bacc("TRN2", target_bir_lowering=False, debug=False, num_devices=8)

    HT = nc.dram_tensor("HT", [D, T], BF16, kind="ExternalInput").ap()
    WqT = nc.dram_tensor("WqT", [D, GD], BF16, kind="ExternalInput").ap()
    WkT = nc.dram_tensor("WkT", [D, GD], BF16, kind="ExternalInput").ap()
    WvS = nc.dram_tensor("WvS", [D, GV], BF16, kind="ExternalInput").ap()
    WoS = nc.dram_tensor("WoS", [GD, D], F32R, kind="ExternalInput").ap()
    bq = nc.dram_tensor("bq", [1, GD], BF16, kind="ExternalInput").ap()
    bk = nc.dram_tensor("bk", [1, GD], BF16, kind="ExternalInput").ap()
    bvS = nc.dram_tensor("bvS", [1, GV], BF16, kind="ExternalInput").ap()
    kpm = nc.dram_tensor("kpm", [128, NT], F32, kind="ExternalInput").ap()
    O = nc.dram_tensor("O", [T, D], BF16, kind="ExternalOutput").ap()

    ENGS = [nc.sync, nc.scalar, nc.gpsimd]

    with tile.TileContext(nc) as tc, ExitStack() as octx:
        cpool = octx.enter_context(tc.tile_pool(name="const", bufs=1))
        keep = octx.enter_context(tc.tile_pool(name="keep", bufs=1))
        sbuf = octx.enter_context(tc.tile_pool(name="work", bufs=1))
        bps = octx.enter_context(tc.tile_pool(name="bps", bufs=1, space="PSUM"))
        aps_ctx = ExitStack()
        aps = aps_ctx.enter_context(tc.tile_pool(name="aps", bufs=1, space="PSUM"))
        bps2 = None  # opened once projection PSUM frees (start of J=3)

        # ---- constants ----
        ones_f = cpool.tile([1, 512], F32, name="ones_f", tag="ones_f")
        nc.vector.memset(ones_f[:], 1.0)
        ones_r = cpool.tile([1, 512], BF16, name="ones_r", tag="ones_r")
        nc.vector.tensor_copy(ones_r[:], ones_f[:])

        bq_r = cpool.tile([1, GD], BF16, name="bq_r", tag="bq_r")
        bk_r = cpool.tile([1, GD], BF16, name="bk_r", tag="bk_r")
        bv_r = cpool.tile([1, GV], BF16, name="bv_r", tag="bv_r")
        kpm_sb = cpool.tile([128, NT], F32, name="kpm_sb", tag="kpm_sb")

        lnk = cpool.tile([128, 1], F32, name="lnk", tag="lnk")
        nc.vector.memset(lnk[:], EXP_LNK)

        # ---- long-lived activations ----
        qT = [keep.tile([128, T], ATT, name=f"qT{m}", tag=f"qT{m}") for m in range(2)]
        kT = [keep.tile([128, T], ATT, name=f"kT{m}", tag=f"kT{m}") for m in range(2)]
        vt = [keep.tile([128, 512], ATT, name=f"vt{t}", tag=f"vt{t}") for t in range(NT)]
        attT = [keep.tile([128, T], F32R, name=f"attT{m}", tag=f"attT{m}") for m in range(2)]
        wo_r = [keep.tile([128, D], F32R, name=f"wo{i}", tag=f"wo{i}") for i in range(2)]
        ht_r = [keep.tile([128, T], BF16, name=f"ht{k}", tag=f"ht{k}") for k in range(NKD)]
        wq_r = sbuf.tile([128, NKD * GD], BF16, name="wq_r", tag="wq_r")
        wk_r = sbuf.tile([128, NKD * GD], BF16, name="wk_r", tag="wk_r")
        wv_r = sbuf.tile([128, NKD * GV], BF16, name="wv_r", tag="wv_r")

        for t in range(NT):
            nc.vector.memset(vt[t][:], 0.0)

        # ---- input DMA, ordered for earliest granule-0 start: the first
        # issue on each engine queue is data the very first matmuls need ----
        for k in range(NKD):
            nc.sync.dma_start(wk_r[:, k * GD:(k + 1) * GD], WkT[k * 128:(k + 1) * 128, :])
            nc.scalar.dma_start(ht_r[k][:, 0:512], HT[k * 128:(k + 1) * 128, 0:512])
            nc.gpsimd.dma_start(wv_r[:, k * GV:(k + 1) * GV], WvS[k * 128:(k + 1) * 128, :])
        nc.sync.dma_start(bk_r[:], bk[:])
        nc.gpsimd.dma_start(bv_r[:], bvS[:])
        nc.gpsimd.dma_start(kpm_sb[:], kpm[:])
        nc.sync.dma_start(bq_r[:], bq[:])
        for k in range(NKD):
            nc.sync.dma_start(wq_r[:, k * GD:(k + 1) * GD], WqT[k * 128:(k + 1) * 128, :])
            ENGS[(k % 2) + 1].dma_start(
                ht_r[k][:, 512:1024], HT[k * 128:(k + 1) * 128, 512:1024]
            )
        for k in range(NKD):
            ENGS[k % 3].dma_start(
                ht_r[k][:, 1024:1536], HT[k * 128:(k + 1) * 128, 1024:1536]
            )
        for i in range(2):
            nc.gpsimd.dma_start(wo_r[i][:], WoS[i * 128:(i + 1) * 128, :])
        for k in range(NKD):
            ENGS[k % 3].dma_start(
                ht_r[k][:, 1536:2048], HT[k * 128:(k + 1) * 128, 1536:2048]
            )

        # ---------- projection granule work (phase A) ----------
        def proj_group(w_r, dest, brow, m, n):
            ps = aps.tile([128, 512], F32, name="ps", tag="aps", bufs=2)
            for k in range(NKD):
                nc.tensor.matmul(
                    ps[:],
                    w_r[:, k * GD + m * 128: k * GD + m * 128 + 128],
                    ht_r[k][:, n * 512:(n + 1) * 512],
                    start=(k == 0), stop=False,
                )
            nc.tensor.matmul(
                ps[:], brow[:, m * 128:(m + 1) * 128], ones_r[:],
                start=False, stop=True,
            )
            nc.vector.tensor_copy(dest[m][:, n * 512:(n + 1) * 512], ps[:])

        def vt_group(t):
            vp = aps.tile([128, 512], F32, name="vp", tag="aps", bufs=2)
            for k in range(NKD):
                nc.tensor.matmul(
                    vp[:, 0:GV],
                    ht_r[k][:, t * 128:(t + 1) * 128],
                    wv_r[:, k * GV:(k + 1) * GV],
                    start=(k == 0), stop=False,
                )
            nc.tensor.matmul(
                vp[:, 0:GV], ones_r[:, 0:128], bv_r[:], start=False, stop=True
            )
            nc.vector.tensor_copy(
                vt[t][:].rearrange("p (h c) -> p h c", c=128)[:, :, 0:65],
                vp[:, 0:GV].rearrange("p (h c) -> p h c", c=65),
            )
            nc.vector.tensor_scalar_mul(vt[t][:], vt[t][:], kpm_sb[:, t:t + 1])

        def a_granule_groups(n):
            gs = []
            for m in range(2):
                gs.append(lambda m=m: proj_group(wk_r, kT, bk_r, m, n))
            for tt in range(4 * n, 4 * n + 4):
                gs.append(lambda tt=tt: vt_group(tt))
            for m in range(2):
                gs.append(lambda m=m: proj_group(wq_r, qT, bq_r, m, n))
            return gs

        # ---------- output projection blocks (phase C) ----------
        c_flip = [0]

        def c_block(t):
            op = bps.tile([128, 1024], F32, name="cop", tag="sc", bufs=2)
            for nn in range(2):
                for hp2 in range(2):
                    nc.tensor.matmul(
                        op[:, nn * 512:(nn + 1) * 512],
                        attT[hp2][:, t * 128:(t + 1) * 128],
                        wo_r[hp2][:, nn * 512:(nn + 1) * 512],
                        start=(hp2 == 0), stop=(hp2 == 1),
                    )
            ot = sbuf.tile([128, D], BF16, name="ot", tag="ot", bufs=3)
            if c_flip[0] % 2 == 0:
                nc.scalar.copy(ot[:], op[:])
            else:
                nc.vector.tensor_copy(ot[:], op[:])
            c_flip[0] += 1
            nc.sync.dma_start(O[t * 128:(t + 1) * 128, :], ot[:])

        # ---------- attention (phase B) with normalize staging ----------
        def issue_srows(at):
            # stage the PSUM denominator rows (partition 64) to SBUF on the
            # scalar engine (DMA cannot read PSUM); issued at block end so
            # the next block's reciprocal finds them ready
            srows = []
            for hh in range(2):
                srow = sbuf.tile([1, 512], F32, name="srow", tag="srow", bufs=4)
                nc.scalar.copy(srow[:], at[hh][64:65, :])
                srows.append(srow)
            return srows

        def normalize_recip(hp, J, at, srows):
            rbs = []
            for hh in range(2):
                rc = sbuf.tile([1, 512], F32, name="rc", tag="rc", bufs=4)
                nc.vector.reciprocal_approx_fast(out=rc[:], in_=srows[hh][:])
                rb = sbuf.tile([64, 512], F32, name="rb", tag="rb", bufs=4)
                nc.gpsimd.partition_broadcast(rb[:], rc[:])
                rbs.append(rb)
            return rbs

        def normalize_mult(hp, J, at, srows, rbs):
            for hh in range(2):
                nc.vector.tensor_tensor(
                    attT[hp][hh * 64:(hh + 1) * 64, J * 512:(J + 1) * 512],
                    at[hh][0:64, :],
                    rbs[hh][:],
                    op=OP.mult,
                )

        # granule 0 runs alone up front
        for g in a_granule_groups(0):
            g()

        a_q = []
        c_q = []
        pending_norm = None
        order = [(0, 0), (0, 1), (1, 0), (1, 1), (2, 0), (2, 1), (3, 0), (3, 1)]
        for bi, (J, hp) in enumerate(order):
            if hp == 0 and J < 3:
                a_q.extend(a_granule_groups(J + 1))
            if bi == 6:
                # all projection work has drained: release its 2 PSUM banks
                # and open a third scores slot, deepening the exp->scores
                # bank-reuse chain from 2 to 3 for the chain-paced J=3 blocks
                assert not a_q
                aps_ctx.close()
                bps2 = octx.enter_context(
                    tc.tile_pool(name="bps2", bufs=1, space="PSUM")
                )
            n_kc = 4 * J + 4
            at = [
                bps.tile([128, 512], F32, name=f"at{hh}", tag="av", bufs=2)
                for hh in range(2)
            ]
            # diagonal first (full width, opens the PSUM accumulation), then
            # off-diagonals, then the narrow diagonals
            kcs = [4 * J] + list(range(4 * J)) + [4 * J + i for i in range(1, 4)]

            def issue_sc_exp(kc, use_dve, use_sc2=False, J=J, hp=hp):
                off = max(0, 128 * (kc - 4 * J))
                w = 512 - off
                if use_sc2:
                    sc = bps2.tile([128, 1024], F32, name="sc2", tag="sc2", bufs=1)
                else:
                    sc = bps.tile([128, 1024], F32, name="sc", tag="sc", bufs=2)
                for hh in range(2):
                    nc.tensor.matmul(
                        sc[:, hh * 512:hh * 512 + w],
                        kT[hp][hh * 64:(hh + 1) * 64, kc * 128:(kc + 1) * 128],
                        qT[hp][hh * 64:(hh + 1) * 64, J * 512 + off:(J + 1) * 512],
                        start=True, stop=True,
                        tile_position=(hh * 64, 0),
                    )
                ex = sbuf.tile([128, 1024], ATT, name="ex", tag="ex", bufs=8)
                if use_dve == "split":
                    # both engines on one chunk (one head each): halves the
                    # exp latency in the sc-bank reuse chain, which is what
                    # paces the big J blocks once projection filler runs out
                    nc.scalar.activation(
                        ex[:, 0:w], sc[:, 0:w], AF.Exp, scale=8.0, bias=lnk[:]
                    )
                    nc.vector._custom_dve(
                        exp8m, out=ex[:, 512:512 + w], in0=sc[:, 512:512 + w],
                        s0=EXP_A2, s1=EXP_A1, imm2=EXP_A0,
                    )
                else:
                    exs = ex[:].rearrange("p (h c) -> p h c", c=512)[:, :, 0:w]
                    scs = sc[:].rearrange("p (h c) -> p h c", c=512)[:, :, 0:w]
                    if use_dve:
                        nc.vector._custom_dve(
                            exp8m, out=exs, in0=scs,
                            s0=EXP_A2, s1=EXP_A1, imm2=EXP_A0,
                        )
                    else:
                        nc.scalar.activation(exs, scs, AF.Exp, scale=8.0, bias=lnk[:])
                if off or kc == 4 * J:
                    # causal mask on the diagonal 128 queries of both heads in
                    # one gpsimd op (head-axis coefficient 0): keep col >= p
                    mk = ex[:].rearrange("p (h c) -> p h c", c=512)[:, :, 0:128]
                    nc.gpsimd.affine_select(
                        out=mk, in_=mk, compare_op=OP.is_ge, fill=0.0,
                        base=0, pattern=[[0, 2], [1, 128]],
                        channel_multiplier=-1,
                    )
                return ex

            def issue_av(kc, ex, first, last, J=J, hp=hp, at=at):
                off = max(0, 128 * (kc - 4 * J))
                w = 512 - off
                for hh in range(2):
                    h = 2 * hp + hh
                    nc.tensor.matmul(
                        at[hh][:, off:512],
                        vt[kc][:, h * 128:(h + 1) * 128],
                        ex[:, hh * 512:hh * 512 + w],
                        start=first, stop=last,
                    )

            # while the previous block's normalize is outstanding this
            # block's AVs are deferred (an early AV would wait on the at-bank
            # release inside the in-order PE queue and deadlock against the
            # normalize chain); once the multiply is issued AVs drain with
            # one chunk of lookahead
            rti = 1 if n_kc == 4 else 2
            pace_a = max(1, (2 * n_kc) // 8)
            avq = []
            rbs_pending = None
            for ti, kc in enumerate(kcs):
                mode = ti % 8 in (2, 5, 7)
                ex = issue_sc_exp(kc, use_dve=mode, use_sc2=(J == 3 and ti % 3 == 2))
                if pending_norm is not None:
                    if ti == rti:
                        rbs_pending = normalize_recip(*pending_norm)
                    if ti == rti + 2:
                        normalize_mult(*pending_norm, rbs_pending)
                        if pending_norm[0] == 1:  # hp==1: query block J done
                            Jd = pending_norm[1]
                            c_q.extend(
                                (lambda t=t: c_block(t))
                                for t in range(4 * Jd, 4 * Jd + 4)
                            )
                        pending_norm = None
                        rbs_pending = None
                avq.append((kc, ex, ti))
                if pending_norm is None:
                    while len(avq) > 1:
                        k0, e0, t0 = avq.pop(0)
                        issue_av(k0, e0, first=(t0 == 0), last=False)
                if a_q and ti % pace_a == 0:
                    a_q.pop(0)()
                if c_q and ti % 3 == 0:
                    c_q.pop(0)()
            while len(avq) > 1:
                k0, e0, t0 = avq.pop(0)
                issue_av(k0, e0, first=(t0 == 0), last=False)
            k0, e0, t0 = avq.pop(0)
            issue_av(k0, e0, first=(t0 == 0), last=True)
            while a_q and hp == 1:
                a_q.pop(0)()
            pending_norm = (hp, J, at, issue_srows(at))

        # tail: the last block's (3,1) normalize is still outstanding, but
        # attT[0] for J=3 is already normalized — run the final C blocks'
        # hp=0 accumulation halves under the normalize chain
        while c_q:
            c_q.pop(0)()
        tails = []
        for t in (12, 13, 14):
            if t == 14:
                op = bps2.tile([128, 1024], F32, name="cop2", tag="sc2", bufs=1)
            else:
                op = bps.tile([128, 1024], F32, name="cop", tag="sc", bufs=2)
            for nn in range(2):
                nc.tensor.matmul(
                    op[:, nn * 512:(nn + 1) * 512],
                    attT[0][:, t * 128:(t + 1) * 128],
                    wo_r[0][:, nn * 512:(nn + 1) * 512],
                    start=True, stop=False,
                )
            tails.append((t, op))
        rbs_pending = normalize_recip(*pending_norm)
        normalize_mult(*pending_norm, rbs_pending)
        for t, op in tails:
            for nn in range(2):
                nc.tensor.matmul(
                    op[:, nn * 512:(nn + 1) * 512],
                    attT[1][:, t * 128:(t + 1) * 128],
                    wo_r[1][:, nn * 512:(nn + 1) * 512],
                    start=False, stop=True,
                )
            ot = sbuf.tile([128, D], BF16, name="ot", tag="ot", bufs=3)
            if t % 2 == 0:
                nc.scalar.copy(ot[:], op[:])
            else:
                nc.vector.tensor_copy(ot[:], op[:])
            nc.sync.dma_start(O[t * 128:(t + 1) * 128, :], ot[:])
        c_block(15)

    nc.compile()
    _NC_CACHE["nc"] = nc
    return nc


def _prep_core_inputs(H, key_padding_mask, Wq, bq, Wk, bk, Wv, bv, Wo, bo):
    keep = 1.0 - np.asarray(key_padding_mask, dtype=np.float32)  # [B, T]
    in_maps = []
    for c in range(8):
        b, g = divmod(c, 4)
        sl = slice(g * GD, (g + 1) * GD)
        WvT = Wv[sl].T  # [D, GD]
        WvS = np.zeros((D, GV), dtype=np.float32)
        bvS = np.zeros((1, GV), dtype=np.float32)
        for h in range(HPC):
            WvS[:, h * 65:h * 65 + 64] = WvT[:, h * 64:(h + 1) * 64]
            bvS[0, h * 65:h * 65 + 64] = bv[sl][h * 64:(h + 1) * 64]
            bvS[0, h * 65 + 64] = 1.0
        bf = ml_dtypes.bfloat16
        # q is scaled by 1/64 so PSUM scores arrive as u = 0.125*r/8, the
        # operating range of the EXP8M cubic; the ACT path applies scale=8.
        in_maps.append({
            "HT": np.ascontiguousarray(H[b].T).astype(bf),
            "WqT": np.ascontiguousarray(Wq[sl].T / 64.0).astype(bf),
            "WkT": np.ascontiguousarray(Wk[sl].T).astype(bf),
            "WvS": WvS.astype(bf),
            "WoS": np.ascontiguousarray(Wo[:, sl].T),
            "bq": np.ascontiguousarray(bq[sl][None, :] / 64.0).astype(bf),
            "bk": np.ascontiguousarray(bk[sl][None, :]).astype(bf),
            "bvS": bvS.astype(bf),
            "kpm": np.ascontiguousarray(keep[b].reshape(NT, 128).T),
        })
    return in_maps


def kernel(H, key_padding_mask, Wq, bq, Wk, bk, Wv, bv, Wo, bo, _run_kwargs=None):
    H = np.asarray(H, dtype=np.float32)
    Wq = np.asarray(Wq, dtype=np.float32)
    Wk = np.asarray(Wk, dtype=np.float32)
    Wv = np.asarray(Wv, dtype=np.float32)
    Wo = np.asarray(Wo, dtype=np.float32)
    bq = np.asarray(bq, dtype=np.float32)
    bk = np.asarray(bk, dtype=np.float32)
    bv = np.asarray(bv, dtype=np.float32)
    bo = np.asarray(bo, dtype=np.float32)

    nc = build()
    in_maps = _prep_core_inputs(H, key_padding_mask, Wq, bq, Wk, bk, Wv, bv, Wo, bo)
    res = bass_utils.run_bass_kernel_spmd(
        nc, in_maps, core_ids=list(range(8)), **(_run_kwargs or {})
    )
    out = np.zeros((B, T, D), dtype=np.float32)
    for c in range(8):
        out[c // 4] += np.asarray(res.results[c]["O"], dtype=np.float32)
    out += bo
    if _run_kwargs:
        kernel.last_result = res
    return out


# revision 57
# speedup vs baseline: 1.1702x; 1.1702x over previous
"""Multi-head self-attention (B=2, T=2048, D=1024, 16 heads) on 8 TRN2 cores.

Sharding: core c = (b, g) with b = c // 4 (batch), g = c % 4 (head group of 4).
Each core computes q/k/v projections for its 4 heads, causal softmax
attention, and a partial output projection (its 256 columns of the
concat-head dim against Wo). Host sums the 4 partials per batch and adds bo.

Single merged pipeline, ordered so the PE never idles (keeps the HAM clock
gate warm): projection "granules" (one 512-token key block: kT, vt, qT) are
interleaved chunk-by-chunk with the attention stream of the previous key
block and with output-projection blocks of the block before that.

  granule n:  kT/qT [256, 512-slice] (transposed projections, head pairs
              stacked on partitions) and vt [512, 260] natural (per head 64
              value cols + a ones col that makes the AV matmul emit softmax
              denominators).
  B(J, hp):   per key-chunk kc: scoresT [128, 2x512] for both heads via
              tile_position row packing; exp on the scalar ACT (Exp LUT,
              scale=8, bias=ln K) or the vector engine (custom EXP8M op:
              monic cubic + 3 squarings, K-scaled) -- both emit
              K*exp(0.125 r) so K cancels in the softmax; causal masks via
              gpsimd affine_select; AV accumulates [v|1].T @ ex into
              at [65, 512] (row 64 = denominators). Normalize: denominators
              staged by scalar copy, reciprocal_approx_fast on DVE, gpsimd
              partition broadcast, DVE multiply into attT -- staged across
              the next block so no in-order engine queue ever stalls.
  C(t):       O[t-block] = attT.T @ WoS accumulated over head pairs in PSUM,
              copied to bf16 and DMA'd out; host converts/sums partials.
"""

import ml_dtypes
import numpy as np

import concourse.bass as bass
import concourse.tile as tile
from concourse import bacc, mybir
from concourse import bass_utils
from concourse import dve_ops
from concourse.dve_ops import DveOp
from concourse.dve_spec import Spec, Src0, C0, C1, C2, sq, lower as dve_lower
from concourse.dve_uop import DveOpSpec
from contextlib import ExitStack

F32 = mybir.dt.float32
F32R = mybir.dt.float32r
BF16 = mybir.dt.bfloat16
ATT = BF16  # dtype for attention-phase matmul operands
AF = mybir.ActivationFunctionType
OP = mybir.AluOpType

B, T, D = 2, 2048, 1024
NH, DH = 16, 64
HPC = 4            # heads per core
GD = HPC * DH      # 256, group dim
GV = HPC * (DH + 1)  # 260, v tile width
NKD = D // 128     # 8 K-chunks for projections
NT = T // 128      # 16 token chunks
NJ = T // 512      # 4 query blocks

_NC_CACHE = {}

# exp via monic cubic + 3 squarings on the vector engine:
#   p(u) = ((u + A2) u + A1) u + A0;  p(u)^8 ~= EXPK * exp(8u)  on |u| <= 0.47
# (max rel err 2.2e-4 in fp32). Scores arrive in PSUM pre-scaled to u = r/64
# (0.125/8 folded into Wq host-side); the ACT path uses scale=8, bias=ln(EXPK)
# so both engines emit EXPK * exp(0.125 r) and EXPK cancels in the softmax.
EXP_A2 = 3.06702906
EXP_A1 = 6.02255865
EXP_A0 = 6.01835402
EXP_LNK = 14.36056232


def _register_exp8m():
    if "EXP8M" in dve_ops._SUB_OPCODE_FOR_NAME:
        return next(o for o in dve_ops.OPS if o.name == "EXP8M")
    _p = ((Src0 + C0) * Src0 + C1) * Src0 + C2
    _body = sq(sq(sq(_p)))

    def _ref(in0, in1, s0, s1, imm2):
        p = (((in0 + s0) * in0 + s1) * in0 + imm2).astype(np.float32)
        return ((p * p) ** 2) ** 2

    spec = Spec(body=_body, reference=_ref)
    ver = "v3"
    sha = DveOpSpec(name="EXP8M", opcode=1, uops=dve_lower(spec, ver=ver),
                    rd1_en=False).sha(ver)
    op = DveOp("EXP8M", spec, subdim=False, uops_sha={ver: sha})
    dve_ops.OPS.append(op)
    dve_ops.CUSTOM_DVE_SPECS[op.name] = op.spec
    dve_ops._SUB_OPCODE_FOR_NAME[op.name] = (
        dve_ops._CUSTOM_DVE_ROW_BASE + len(dve_ops.OPS) - 1
    )
    return op


def build():
    if "nc" in _NC_CACHE:
        return _NC_CACHE["nc"]
    exp8m = _register_exp8m()
    nc = bacc.Bacc("TRN2", target_bir_lowering=False, debug=False, num_devices=8)

    HT = nc.dram_tensor("HT", [D, T], BF16, kind="ExternalInput").ap()
    WqT = nc.dram_tensor("WqT", [D, GD], BF16, kind="ExternalInput").ap()
    WkT = nc.dram_tensor("WkT", [D, GD], BF16, kind="ExternalInput").ap()
    WvS = nc.dram_tensor("WvS", [D, GV], BF16, kind="ExternalInput").ap()
    WoS = nc.dram_tensor("WoS", [GD, D], F32R, kind="ExternalInput").ap()
    bq = nc.dram_tensor("bq", [1, GD], BF16, kind="ExternalInput").ap()
    bk = nc.dram_tensor("bk", [1, GD], BF16, kind="ExternalInput").ap()
    bvS = nc.dram_tensor("bvS", [1, GV], BF16, kind="ExternalInput").ap()
    kpm = nc.dram_tensor("kpm", [128, NT], F32, kind="ExternalInput").ap()
    O = nc.dram_tensor("O", [T, D], BF16, kind="ExternalOutput").ap()

    ENGS = [nc.sync, nc.scalar, nc.gpsimd]

    with tile.TileContext(nc) as tc, ExitStack() as octx:
        cpool = octx.enter_context(tc.tile_pool(name="const", bufs=1))
        keep = octx.enter_context(tc.tile_pool(name="keep", bufs=1))
        sbuf = octx.enter_context(tc.tile_pool(name="work", bufs=1))
        bps = octx.enter_context(tc.tile_pool(name="bps", bufs=1, space="PSUM"))
        aps_ctx = ExitStack()
        aps = aps_ctx.enter_context(tc.tile_pool(name="aps", bufs=1, space="PSUM"))
        bps2 = None  # opened once projection PSUM frees (start of J=3)

        # ---- constants ----
        ones_f = cpool.tile([1, 512], F32, name="ones_f", tag="ones_f")
        nc.vector.memset(ones_f[:], 1.0)
        ones_r = cpool.tile([1, 512], BF16, name="ones_r", tag="ones_r")
        nc.vector.tensor_copy(ones_r[:], ones_f[:])

        bq_r = cpool.tile([1, GD], BF16, name="bq_r", tag="bq_r")
        bk_r = cpool.tile([1, GD], BF16, name="bk_r", tag="bk_r")
        bv_r = cpool.tile([1, GV], BF16, name="bv_r", tag="bv_r")
        kpm_sb = cpool.tile([128, NT], F32, name="kpm_sb", tag="kpm_sb")

        lnk = cpool.tile([128, 1], F32, name="lnk", tag="lnk")
        nc.vector.memset(lnk[:], EXP_LNK)

        # ---- long-lived activations ----
        qT = [keep.tile([128, T], ATT, name=f"qT{m}", tag=f"qT{m}") for m in range(2)]
        kT = [keep.tile([128, T], ATT, name=f"kT{m}", tag=f"kT{m}") for m in range(2)]
        vt = [keep.tile([128, 512], ATT, name=f"vt{t}", tag=f"vt{t}") for t in range(NT)]
        attT = [keep.tile([128, T], F32R, name=f"attT{m}", tag=f"attT{m}") for m in range(2)]
        wo_r = [keep.tile([128, D], F32R, name=f"wo{i}", tag=f"wo{i}") for i in range(2)]
        ht_r = [keep.tile([128, T], BF16, name=f"ht{k}", tag=f"ht{k}") for k in range(NKD)]
        wq_r = sbuf.tile([128, NKD * GD], BF16, name="wq_r", tag="wq_r")
        wk_r = sbuf.tile([128, NKD * GD], BF16, name="wk_r", tag="wk_r")
        wv_r = sbuf.tile([128, NKD * GV], BF16, name="wv_r", tag="wv_r")

        for t in range(NT):
            nc.vector.memset(vt[t][:], 0.0)

        # ---- input DMA, ordered for earliest granule-0 start: the first
        # issue on each engine queue is data the very first matmuls need ----
        for k in range(NKD):
            nc.sync.dma_start(wk_r[:, k * GD:(k + 1) * GD], WkT[k * 128:(k + 1) * 128, :])
            nc.scalar.dma_start(ht_r[k][:, 0:512], HT[k * 128:(k + 1) * 128, 0:512])
            nc.gpsimd.dma_start(wv_r[:, k * GV:(k + 1) * GV], WvS[k * 128:(k + 1) * 128, :])
        nc.sync.dma_start(bk_r[:], bk[:])
        nc.gpsimd.dma_start(bv_r[:], bvS[:])
        nc.gpsimd.dma_start(kpm_sb[:], kpm[:])
        nc.sync.dma_start(bq_r[:], bq[:])
        for k in range(NKD):
            nc.sync.dma_start(wq_r[:, k * GD:(k + 1) * GD], WqT[k * 128:(k + 1) * 128, :])
            nc.scalar.dma_start(ht_r[k][:, 512:1024], HT[k * 128:(k + 1) * 128, 512:1024])
            nc.gpsimd.dma_start(ht_r[k][:, 1024:1536], HT[k * 128:(k + 1) * 128, 1024:1536])
        for i in range(2):
            nc.gpsimd.dma_start(wo_r[i][:], WoS[i * 128:(i + 1) * 128, :])
        for k in range(NKD):
            ENGS[k % 3].dma_start(
                ht_r[k][:, 1536:2048], HT[k * 128:(k + 1) * 128, 1536:2048]
            )

        # ---------- projection granule work (phase A) ----------
        def proj_group(w_r, dest, brow, m, n):
            ps = aps.tile([128, 512], F32, name="ps", tag="aps", bufs=2)
            for k in range(NKD):
                nc.tensor.matmul(
                    ps[:],
                    w_r[:, k * GD + m * 128: k * GD + m * 128 + 128],
                    ht_r[k][:, n * 512:(n + 1) * 512],
                    start=(k == 0), stop=False,
                )
            nc.tensor.matmul(
                ps[:], brow[:, m * 128:(m + 1) * 128], ones_r[:],
                start=False, stop=True,
            )
            nc.vector.tensor_copy(dest[m][:, n * 512:(n + 1) * 512], ps[:])

        def vt_group(t):
            vp = aps.tile([128, 512], F32, name="vp", tag="aps", bufs=2)
            for k in range(NKD):
                nc.tensor.matmul(
                    vp[:, 0:GV],
                    ht_r[k][:, t * 128:(t + 1) * 128],
                    wv_r[:, k * GV:(k + 1) * GV],
                    start=(k == 0), stop=False,
                )
            nc.tensor.matmul(
                vp[:, 0:GV], ones_r[:, 0:128], bv_r[:], start=False, stop=True
            )
            nc.vector.tensor_copy(
                vt[t][:].rearrange("p (h c) -> p h c", c=128)[:, :, 0:65],
                vp[:, 0:GV].rearrange("p (h c) -> p h c", c=65),
            )
            nc.vector.tensor_scalar_mul(vt[t][:], vt[t][:], kpm_sb[:, t:t + 1])

        def a_granule_groups(n):
            gs = []
            for m in range(2):
                gs.append(lambda m=m: proj_group(wk_r, kT, bk_r, m, n))
            for tt in range(4 * n, 4 * n + 4):
                gs.append(lambda tt=tt: vt_group(tt))
            for m in range(2):
                gs.append(lambda m=m: proj_group(wq_r, qT, bq_r, m, n))
            return gs

        # ---------- output projection blocks (phase C) ----------
        c_flip = [0]

        def c_block(t):
            op = bps.tile([128, 1024], F32, name="cop", tag="sc", bufs=2)
            for nn in range(2):
                for hp2 in range(2):
                    nc.tensor.matmul(
                        op[:, nn * 512:(nn + 1) * 512],
                        attT[hp2][:, t * 128:(t + 1) * 128],
                        wo_r[hp2][:, nn * 512:(nn + 1) * 512],
                        start=(hp2 == 0), stop=(hp2 == 1),
                    )
            ot = sbuf.tile([128, D], BF16, name="ot", tag="ot", bufs=3)
            if c_flip[0] % 2 == 0:
                nc.scalar.copy(ot[:], op[:])
            else:
                nc.vector.tensor_copy(ot[:], op[:])
            c_flip[0] += 1
            nc.sync.dma_start(O[t * 128:(t + 1) * 128, :], ot[:])

        # ---------- attention (phase B) with normalize staging ----------
        def issue_srows(at):
            # stage the PSUM denominator rows (partition 64) to SBUF on the
            # scalar engine (DMA cannot read PSUM); issued at block end so
            # the next block's reciprocal finds them ready
            srows = []
            for hh in range(2):
                srow = sbuf.tile([1, 512], F32, name="srow", tag="srow", bufs=4)
                nc.scalar.copy(srow[:], at[hh][64:65, :])
                srows.append(srow)
            return srows

        def normalize_recip(hp, J, at, srows):
            rbs = []
            for hh in range(2):
                rc = sbuf.tile([1, 512], F32, name="rc", tag="rc", bufs=4)
                nc.vector.reciprocal_approx_fast(out=rc[:], in_=srows[hh][:])
                rb = sbuf.tile([64, 512], F32, name="rb", tag="rb", bufs=4)
                nc.gpsimd.partition_broadcast(rb[:], rc[:])
                rbs.append(rb)
            return rbs

        def normalize_mult(hp, J, at, srows, rbs):
            for hh in range(2):
                nc.vector.tensor_tensor(
                    attT[hp][hh * 64:(hh + 1) * 64, J * 512:(J + 1) * 512],
                    at[hh][0:64, :],
                    rbs[hh][:],
                    op=OP.mult,
                )

        # granule 0 runs alone up front
        for g in a_granule_groups(0):
            g()

        a_q = []
        c_q = []
        pending_norm = None
        order = [(0, 0), (0, 1), (1, 0), (1, 1), (2, 0), (2, 1), (3, 0), (3, 1)]
        for bi, (J, hp) in enumerate(order):
            if hp == 0 and J < 3:
                a_q.extend(a_granule_groups(J + 1))
            if bi == 6:
                # all projection work has drained: release its 2 PSUM banks
                # and open a third scores slot, deepening the exp->scores
                # bank-reuse chain from 2 to 3 for the chain-paced J=3 blocks
                assert not a_q
                aps_ctx.close()
                bps2 = octx.enter_context(
                    tc.tile_pool(name="bps2", bufs=1, space="PSUM")
                )
            n_kc = 4 * J + 4
            at = [
                bps.tile([128, 512], F32, name=f"at{hh}", tag="av", bufs=2)
                for hh in range(2)
            ]
            # diagonal first (full width, opens the PSUM accumulation), then
            # off-diagonals, then the narrow diagonals
            kcs = [4 * J] + list(range(4 * J)) + [4 * J + i for i in range(1, 4)]

            def issue_sc_exp(kc, use_dve, use_sc2=False, J=J, hp=hp):
                off = max(0, 128 * (kc - 4 * J))
                w = 512 - off
                if use_sc2:
                    sc = bps2.tile([128, 1024], F32, name="sc2", tag="sc2", bufs=1)
                else:
                    sc = bps.tile([128, 1024], F32, name="sc", tag="sc", bufs=2)
                for hh in range(2):
                    nc.tensor.matmul(
                        sc[:, hh * 512:hh * 512 + w],
                        kT[hp][hh * 64:(hh + 1) * 64, kc * 128:(kc + 1) * 128],
                        qT[hp][hh * 64:(hh + 1) * 64, J * 512 + off:(J + 1) * 512],
                        start=True, stop=True,
                        tile_position=(hh * 64, 0),
                    )
                ex = sbuf.tile([128, 1024], ATT, name="ex", tag="ex", bufs=8)
                if use_dve == "split":
                    # both engines on one chunk (one head each): halves the
                    # exp latency in the sc-bank reuse chain, which is what
                    # paces the big J blocks once projection filler runs out
                    nc.scalar.activation(
                        ex[:, 0:w], sc[:, 0:w], AF.Exp, scale=8.0, bias=lnk[:]
                    )
                    nc.vector._custom_dve(
                        exp8m, out=ex[:, 512:512 + w], in0=sc[:, 512:512 + w],
                        s0=EXP_A2, s1=EXP_A1, imm2=EXP_A0,
                    )
                else:
                    exs = ex[:].rearrange("p (h c) -> p h c", c=512)[:, :, 0:w]
                    scs = sc[:].rearrange("p (h c) -> p h c", c=512)[:, :, 0:w]
                    if use_dve:
                        nc.vector._custom_dve(
                            exp8m, out=exs, in0=scs,
                            s0=EXP_A2, s1=EXP_A1, imm2=EXP_A0,
                        )
                    else:
                        nc.scalar.activation(exs, scs, AF.Exp, scale=8.0, bias=lnk[:])
                if off or kc == 4 * J:
                    # causal mask on the diagonal 128 queries of both heads in
                    # one gpsimd op (head-axis coefficient 0): keep col >= p
                    mk = ex[:].rearrange("p (h c) -> p h c", c=512)[:, :, 0:128]
                    nc.gpsimd.affine_select(
                        out=mk, in_=mk, compare_op=OP.is_ge, fill=0.0,
                        base=0, pattern=[[0, 2], [1, 128]],
                        channel_multiplier=-1,
                    )
                return ex

            def issue_av(kc, ex, first, last, J=J, hp=hp, at=at):
                off = max(0, 128 * (kc - 4 * J))
                w = 512 - off
                for hh in range(2):
                    h = 2 * hp + hh
                    nc.tensor.matmul(
                        at[hh][:, off:512],
                        vt[kc][:, h * 128:(h + 1) * 128],
                        ex[:, hh * 512:hh * 512 + w],
                        start=first, stop=last,
                    )

            # while the previous block's normalize is outstanding this
            # block's AVs are deferred (an early AV would wait on the at-bank
            # release inside the in-order PE queue and deadlock against the
            # normalize chain); once the multiply is issued AVs drain with
            # one chunk of lookahead
            rti = 1 if n_kc == 4 else 2
            pace_a = max(1, (2 * n_kc) // 8)
            avq = []
            rbs_pending = None
            for ti, kc in enumerate(kcs):
                mode = ti % 8 in (2, 5, 7)
                ex = issue_sc_exp(kc, use_dve=mode, use_sc2=(J == 3 and ti % 3 == 2))
                if pending_norm is not None:
                    if ti == rti:
                        rbs_pending = normalize_recip(*pending_norm)
                    if ti == rti + 2:
                        normalize_mult(*pending_norm, rbs_pending)
                        if pending_norm[0] == 1:  # hp==1: query block J done
                            Jd = pending_norm[1]
                            c_q.extend(
                                (lambda t=t: c_block(t))
                                for t in range(4 * Jd, 4 * Jd + 4)
                            )
                        pending_norm = None
                        rbs_pending = None
                avq.append((kc, ex, ti))
                if pending_norm is None:
                    while len(avq) > 1:
                        k0, e0, t0 = avq.pop(0)
                        issue_av(k0, e0, first=(t0 == 0), last=False)
                if a_q and ti % pace_a == 0:
                    a_q.pop(0)()
                if c_q and ti % 3 == 0:
                    c_q.pop(0)()
            while len(avq) > 1:
                k0, e0, t0 = avq.pop(0)
                issue_av(k0, e0, first=(t0 == 0), last=False)
            k0, e0, t0 = avq.pop(0)
            issue_av(k0, e0, first=(t0 == 0), last=True)
            while a_q and hp == 1:
                a_q.pop(0)()
            pending_norm = (hp, J, at, issue_srows(at))

        # tail: the last block's (3,1) normalize is still outstanding, but
        # attT[0] for J=3 is already normalized — run the final C blocks'
        # hp=0 accumulation halves under the normalize chain
        while c_q:
            c_q.pop(0)()
        tails = []
        for t in (12, 13, 14):
            if t == 14:
                op = bps2.tile([128, 1024], F32, name="cop2", tag="sc2", bufs=1)
            else:
                op = bps.tile([128, 1024], F32, name="cop", tag="sc", bufs=2)
            for nn in range(2):
                nc.tensor.matmul(
                    op[:, nn * 512:(nn + 1) * 512],
                    attT[0][:, t * 128:(t + 1) * 128],
                    wo_r[0][:, nn * 512:(nn + 1) * 512],
                    start=True, stop=False,
                )
            tails.append((t, op))
        rbs_pending = normalize_recip(*pending_norm)
        normalize_mult(*pending_norm, rbs_pending)
        for t, op in tails:
            for nn in range(2):
                nc.tensor.matmul(
                    op[:, nn * 512:(nn + 1) * 512],
                    attT[1][:, t * 128:(t + 1) * 128],
                    wo_r[1][:, nn * 512:(nn + 1) * 512],
                    start=False, stop=True,
                )
            ot = sbuf.tile([128, D], BF16, name="ot", tag="ot", bufs=3)
            if t % 2 == 0:
                nc.scalar.copy(ot[:], op[:])
            else:
                nc.vector.tensor_copy(ot[:], op[:])
            nc.sync.dma_start(O[t * 128:(t + 1) * 128, :], ot[:])
        c_block(15)

    nc.compile()
    _NC_CACHE["nc"] = nc
    return nc


def _prep_core_inputs(H, key_padding_mask, Wq, bq, Wk, bk, Wv, bv, Wo, bo):
    keep = 1.0 - np.asarray(key_padding_mask, dtype=np.float32)  # [B, T]
    in_maps = []
    for c in range(8):
        b, g = divmod(c, 4)
        sl = slice(g * GD, (g + 1) * GD)
        WvT = Wv[sl].T  # [D, GD]
        WvS = np.zeros((D, GV), dtype=np.float32)
        bvS = np.zeros((1, GV), dtype=np.float32)
        for h in range(HPC):
            WvS[:, h * 65:h * 65 + 64] = WvT[:, h * 64:(h + 1) * 64]
            bvS[0, h * 65:h * 65 + 64] = bv[sl][h * 64:(h + 1) * 64]
            bvS[0, h * 65 + 64] = 1.0
        bf = ml_dtypes.bfloat16
        # q is scaled by 1/64 so PSUM scores arrive as u = 0.125*r/8, the
        # operating range of the EXP8M cubic; the ACT path applies scale=8.
        in_maps.append({
            "HT": np.ascontiguousarray(H[b].T).astype(bf),
            "WqT": np.ascontiguousarray(Wq[sl].T / 64.0).astype(bf),
            "WkT": np.ascontiguousarray(Wk[sl].T).astype(bf),
            "WvS": WvS.astype(bf),
            "WoS": np.ascontiguousarray(Wo[:, sl].T),
            "bq": np.ascontiguousarray(bq[sl][None, :] / 64.0).astype(bf),
            "bk": np.ascontiguousarray(bk[sl][None, :]).astype(bf),
            "bvS": bvS.astype(bf),
            "kpm": np.ascontiguousarray(keep[b].reshape(NT, 128).T),
        })
    return in_maps


def kernel(H, key_padding_mask, Wq, bq, Wk, bk, Wv, bv, Wo, bo, _run_kwargs=None):
    H = np.asarray(H, dtype=np.float32)
    Wq = np.asarray(Wq, dtype=np.float32)
    Wk = np.asarray(Wk, dtype=np.float32)
    Wv = np.asarray(Wv, dtype=np.float32)
    Wo = np.asarray(Wo, dtype=np.float32)
    bq = np.asarray(bq, dtype=np.float32)
    bk = np.asarray(bk, dtype=np.float32)
    bv = np.asarray(bv, dtype=np.float32)
    bo = np.asarray(bo, dtype=np.float32)

    nc = build()
    in_maps = _prep_core_inputs(H, key_padding_mask, Wq, bq, Wk, bk, Wv, bv, Wo, bo)
    res = bass_utils.run_bass_kernel_spmd(
        nc, in_maps, core_ids=list(range(8)), **(_run_kwargs or {})
    )
    out = np.zeros((B, T, D), dtype=np.float32)
    for c in range(8):
        out[c // 4] += np.asarray(res.results[c]["O"], dtype=np.float32)
    out += bo
    if _run_kwargs:
        kernel.last_result = res
    return out


# revision 64
# speedup vs baseline: 1.1805x; 1.0088x over previous
"""Multi-head self-attention (B=2, T=2048, D=1024, 16 heads) on 8 TRN2 cores.

Sharding: core c = (b, g) with b = c // 4 (batch), g = c % 4 (head group of 4).
Each core computes q/k/v projections for its 4 heads, causal softmax
attention, and a partial output projection (its 256 columns of the
concat-head dim against Wo). Host sums the 4 partials per batch and adds bo.

Single merged pipeline, ordered so the PE never idles (keeps the HAM clock
gate warm): projection "granules" (one 512-token key block: kT, vt, qT) are
interleaved chunk-by-chunk with the attention stream of the previous key
block and with output-projection blocks of the block before that.

  granule n:  kT/qT [256, 512-slice] (transposed projections, head pairs
              stacked on partitions) and vt [512, 260] natural (per head 64
              value cols + a ones col that makes the AV matmul emit softmax
              denominators).
  B(J, hp):   per key-chunk kc: scoresT [128, 2x512] for both heads via
              tile_position row packing; exp on the scalar ACT (Exp LUT,
              scale=8, bias=ln K) or the vector engine (custom EXP8M op:
              monic cubic + 3 squarings, K-scaled) -- both emit
              K*exp(0.125 r) so K cancels in the softmax; causal masks via
              gpsimd affine_select; AV accumulates [v|1].T @ ex into
              at [65, 512] (row 64 = denominators). Normalize: denominators
              staged by scalar copy, reciprocal_approx_fast on DVE, gpsimd
              partition broadcast, DVE multiply into attT -- staged across
              the next block so no in-order engine queue ever stalls.
  C(t):       O[t-block] = attT.T @ WoS accumulated over head pairs in PSUM,
              copied to bf16 and DMA'd out; host converts/sums partials.
"""

import ml_dtypes
import numpy as np

import concourse.bass as bass
import concourse.tile as tile
from concourse import bacc, mybir
from concourse import bass_utils
from concourse import dve_ops
from concourse.dve_ops import DveOp
from concourse.dve_spec import Spec, Src0, C0, C1, C2, sq, lower as dve_lower
from concourse.dve_uop import DveOpSpec
from contextlib import ExitStack

F32 = mybir.dt.float32
F32R = mybir.dt.float32r
BF16 = mybir.dt.bfloat16
ATT = BF16  # dtype for attention-phase matmul operands
AF = mybir.ActivationFunctionType
OP = mybir.AluOpType

B, T, D = 2, 2048, 1024
NH, DH = 16, 64
HPC = 4            # heads per core
GD = HPC * DH      # 256, group dim
GV = HPC * (DH + 1)  # 260, v tile width
NKD = D // 128     # 8 K-chunks for projections
NT = T // 128      # 16 token chunks
NJ = T // 512      # 4 query blocks

_NC_CACHE = {}

# exp via monic cubic + 3 squarings on the vector engine:
#   p(u) = ((u + A2) u + A1) u + A0;  p(u)^8 ~= EXPK * exp(8u)  on |u| <= 0.47
# (max rel err 2.2e-4 in fp32). Scores arrive in PSUM pre-scaled to u = r/64
# (0.125/8 folded into Wq host-side); the ACT path uses scale=8, bias=ln(EXPK)
# so both engines emit EXPK * exp(0.125 r) and EXPK cancels in the softmax.
EXP_A2 = 3.06702906
EXP_A1 = 6.02255865
EXP_A0 = 6.01835402
EXP_LNK = 14.36056232


def _register_exp8m():
    if "EXP8M" in dve_ops._SUB_OPCODE_FOR_NAME:
        return next(o for o in dve_ops.OPS if o.name == "EXP8M")
    _p = ((Src0 + C0) * Src0 + C1) * Src0 + C2
    _body = sq(sq(sq(_p)))

    def _ref(in0, in1, s0, s1, imm2):
        p = (((in0 + s0) * in0 + s1) * in0 + imm2).astype(np.float32)
        return ((p * p) ** 2) ** 2

    spec = Spec(body=_body, reference=_ref)
    ver = "v3"
    sha = DveOpSpec(name="EXP8M", opcode=1, uops=dve_lower(spec, ver=ver),
                    rd1_en=False).sha(ver)
    op = DveOp("EXP8M", spec, subdim=False, uops_sha={ver: sha})
    dve_ops.OPS.append(op)
    dve_ops.CUSTOM_DVE_SPECS[op.name] = op.spec
    dve_ops._SUB_OPCODE_FOR_NAME[op.name] = (
        dve_ops._CUSTOM_DVE_ROW_BASE + len(dve_ops.OPS) - 1
    )
    return op


def build(skip_qk_bias=False, skip_kpm=False):
    key = ("nc", skip_qk_bias, skip_kpm)
    if key in _NC_CACHE:
        return _NC_CACHE[key]
    exp8m = _register_exp8m()
    nc = bacc.Bacc("TRN2", target_bir_lowering=False, debug=False, num_devices=8)

    HT = nc.dram_tensor("HT", [D, T], BF16, kind="ExternalInput").ap()
    WqT = nc.dram_tensor("WqT", [D, GD], BF16, kind="ExternalInput").ap()
    WkT = nc.dram_tensor("WkT", [D, GD], BF16, kind="ExternalInput").ap()
    WvS = nc.dram_tensor("WvS", [D, GV], BF16, kind="ExternalInput").ap()
    WoS = nc.dram_tensor("WoS", [GD, D], F32R, kind="ExternalInput").ap()
    bq = nc.dram_tensor("bq", [1, GD], BF16, kind="ExternalInput").ap()
    bk = nc.dram_tensor("bk", [1, GD], BF16, kind="ExternalInput").ap()
    bvS = nc.dram_tensor("bvS", [1, GV], BF16, kind="ExternalInput").ap()
    kpm = nc.dram_tensor("kpm", [128, NT], F32, kind="ExternalInput").ap()
    O = nc.dram_tensor("O", [T, D], BF16, kind="ExternalOutput").ap()

    ENGS = [nc.sync, nc.scalar, nc.gpsimd]

    with tile.TileContext(nc) as tc, ExitStack() as octx:
        cpool = octx.enter_context(tc.tile_pool(name="const", bufs=1))
        keep = octx.enter_context(tc.tile_pool(name="keep", bufs=1))
        sbuf = octx.enter_context(tc.tile_pool(name="work", bufs=1))
        bps = octx.enter_context(tc.tile_pool(name="bps", bufs=1, space="PSUM"))
        aps_ctx = ExitStack()
        aps = aps_ctx.enter_context(tc.tile_pool(name="aps", bufs=1, space="PSUM"))
        bps2 = None  # opened once projection PSUM frees (start of J=3)

        # ---- constants ----
        ones_f = cpool.tile([1, 512], F32, name="ones_f", tag="ones_f")
        nc.vector.memset(ones_f[:], 1.0)
        ones_r = cpool.tile([1, 512], BF16, name="ones_r", tag="ones_r")
        nc.vector.tensor_copy(ones_r[:], ones_f[:])

        bq_r = cpool.tile([1, GD], BF16, name="bq_r", tag="bq_r")
        bk_r = cpool.tile([1, GD], BF16, name="bk_r", tag="bk_r")
        bv_r = cpool.tile([1, GV], BF16, name="bv_r", tag="bv_r")
        kpm_sb = cpool.tile([128, NT], F32, name="kpm_sb", tag="kpm_sb")

        lnk = cpool.tile([128, 1], F32, name="lnk", tag="lnk")
        nc.vector.memset(lnk[:], EXP_LNK)

        # ---- long-lived activations ----
        qT = [keep.tile([128, T], ATT, name=f"qT{m}", tag=f"qT{m}") for m in range(2)]
        kT = [keep.tile([128, T], ATT, name=f"kT{m}", tag=f"kT{m}") for m in range(2)]
        vt = [keep.tile([128, 512], ATT, name=f"vt{t}", tag=f"vt{t}") for t in range(NT)]
        attT = [keep.tile([128, T], F32R, name=f"attT{m}", tag=f"attT{m}") for m in range(2)]
        wo_r = [keep.tile([128, D], F32R, name=f"wo{i}", tag=f"wo{i}") for i in range(2)]
        ht_r = [keep.tile([128, T], BF16, name=f"ht{k}", tag=f"ht{k}") for k in range(NKD)]
        wq_r = sbuf.tile([128, NKD * GD], BF16, name="wq_r", tag="wq_r")
        wk_r = sbuf.tile([128, NKD * GD], BF16, name="wk_r", tag="wk_r")
        wv_r = sbuf.tile([128, NKD * GV], BF16, name="wv_r", tag="wv_r")

        for t in range(NT):
            nc.vector.memset(vt[t][:], 0.0)

        # ---- input DMA, ordered for earliest granule-0 start: the first
        # issue on each engine queue is data the very first matmuls need ----
        for k in range(NKD):
            nc.sync.dma_start(wk_r[:, k * GD:(k + 1) * GD], WkT[k * 128:(k + 1) * 128, :])
            nc.scalar.dma_start(ht_r[k][:, 0:512], HT[k * 128:(k + 1) * 128, 0:512])
            nc.gpsimd.dma_start(wv_r[:, k * GV:(k + 1) * GV], WvS[k * 128:(k + 1) * 128, :])
        if not skip_qk_bias:
            nc.sync.dma_start(bk_r[:], bk[:])
        nc.gpsimd.dma_start(bv_r[:], bvS[:])
        if not skip_kpm:
            nc.gpsimd.dma_start(kpm_sb[:], kpm[:])
        if not skip_qk_bias:
            nc.sync.dma_start(bq_r[:], bq[:])
        for k in range(NKD):
            nc.sync.dma_start(wq_r[:, k * GD:(k + 1) * GD], WqT[k * 128:(k + 1) * 128, :])
            nc.scalar.dma_start(ht_r[k][:, 512:1024], HT[k * 128:(k + 1) * 128, 512:1024])
            nc.gpsimd.dma_start(ht_r[k][:, 1024:1536], HT[k * 128:(k + 1) * 128, 1024:1536])
        for i in range(2):
            nc.gpsimd.dma_start(wo_r[i][:], WoS[i * 128:(i + 1) * 128, :])
        for k in range(NKD):
            ENGS[k % 3].dma_start(
                ht_r[k][:, 1536:2048], HT[k * 128:(k + 1) * 128, 1536:2048]
            )

        # ---------- projection granule work (phase A) ----------
        def proj_group(w_r, dest, brow, m, n):
            ps = aps.tile([128, 512], F32, name="ps", tag="aps", bufs=2)
            for k in range(NKD):
                nc.tensor.matmul(
                    ps[:],
                    w_r[:, k * GD + m * 128: k * GD + m * 128 + 128],
                    ht_r[k][:, n * 512:(n + 1) * 512],
                    start=(k == 0),
                    stop=(skip_qk_bias and k == NKD - 1),
                )
            if not skip_qk_bias:
                nc.tensor.matmul(
                    ps[:], brow[:, m * 128:(m + 1) * 128], ones_r[:],
                    start=False, stop=True,
                )
            nc.vector.tensor_copy(dest[m][:, n * 512:(n + 1) * 512], ps[:])

        def vt_group(t):
            vp = aps.tile([128, 512], F32, name="vp", tag="aps", bufs=2)
            for k in range(NKD):
                nc.tensor.matmul(
                    vp[:, 0:GV],
                    ht_r[k][:, t * 128:(t + 1) * 128],
                    wv_r[:, k * GV:(k + 1) * GV],
                    start=(k == 0), stop=False,
                )
            nc.tensor.matmul(
                vp[:, 0:GV], ones_r[:, 0:128], bv_r[:], start=False, stop=True
            )
            nc.vector.tensor_copy(
                vt[t][:].rearrange("p (h c) -> p h c", c=128)[:, :, 0:65],
                vp[:, 0:GV].rearrange("p (h c) -> p h c", c=65),
            )
            if not skip_kpm:
                nc.vector.tensor_scalar_mul(vt[t][:], vt[t][:], kpm_sb[:, t:t + 1])

        def a_granule_groups(n):
            gs = []
            for m in range(2):
                gs.append(lambda m=m: proj_group(wk_r, kT, bk_r, m, n))
            for tt in range(4 * n, 4 * n + 4):
                gs.append(lambda tt=tt: vt_group(tt))
            for m in range(2):
                gs.append(lambda m=m: proj_group(wq_r, qT, bq_r, m, n))
            return gs

        # ---------- output projection blocks (phase C) ----------
        c_flip = [0]

        def c_block(t):
            op = bps.tile([128, 1024], F32, name="cop", tag="sc", bufs=2)
            for nn in range(2):
                for hp2 in range(2):
                    nc.tensor.matmul(
                        op[:, nn * 512:(nn + 1) * 512],
                        attT[hp2][:, t * 128:(t + 1) * 128],
                        wo_r[hp2][:, nn * 512:(nn + 1) * 512],
                        start=(hp2 == 0), stop=(hp2 == 1),
                    )
            ot = sbuf.tile([128, D], BF16, name="ot", tag="ot", bufs=3)
            if c_flip[0] % 2 == 0:
                nc.scalar.copy(ot[:], op[:])
            else:
                nc.vector.tensor_copy(ot[:], op[:])
            c_flip[0] += 1
            nc.sync.dma_start(O[t * 128:(t + 1) * 128, :], ot[:])

        # ---------- attention (phase B) with normalize staging ----------
        def issue_srows(at):
            # stage the PSUM denominator rows (partition 64) to SBUF on the
            # scalar engine (DMA cannot read PSUM); issued at block end so
            # the next block's reciprocal finds them ready
            srows = []
            for hh in range(2):
                srow = sbuf.tile([1, 512], F32, name="srow", tag="srow", bufs=4)
                nc.scalar.copy(srow[:], at[hh][64:65, :])
                srows.append(srow)
            return srows

        def normalize_recip(hp, J, at, srows):
            rbs = []
            for hh in range(2):
                rc = sbuf.tile([1, 512], F32, name="rc", tag="rc", bufs=4)
                nc.vector.reciprocal_approx_fast(out=rc[:], in_=srows[hh][:])
                rb = sbuf.tile([64, 512], F32, name="rb", tag="rb", bufs=4)
                nc.gpsimd.partition_broadcast(rb[:], rc[:])
                rbs.append(rb)
            return rbs

        def normalize_mult(hp, J, at, srows, rbs):
            for hh in range(2):
                nc.vector.tensor_tensor(
                    attT[hp][hh * 64:(hh + 1) * 64, J * 512:(J + 1) * 512],
                    at[hh][0:64, :],
                    rbs[hh][:],
                    op=OP.mult,
                )

        # granule 0 runs alone up front
        for g in a_granule_groups(0):
            g()

        a_q = []
        c_q = []
        pending_norm = None
        order = [(0, 0), (0, 1), (1, 0), (1, 1), (2, 0), (2, 1), (3, 0), (3, 1)]
        for bi, (J, hp) in enumerate(order):
            if hp == 0 and J < 3:
                a_q.extend(a_granule_groups(J + 1))
            if bi == 6:
                # all projection work has drained: release its 2 PSUM banks
                # and open a third scores slot, deepening the exp->scores
                # bank-reuse chain from 2 to 3 for the chain-paced J=3 blocks
                assert not a_q
                aps_ctx.close()
                bps2 = octx.enter_context(
                    tc.tile_pool(name="bps2", bufs=1, space="PSUM")
                )
            n_kc = 4 * J + 4
            at = [
                bps.tile([128, 512], F32, name=f"at{hh}", tag="av", bufs=2)
                for hh in range(2)
            ]
            # diagonal first (full width, opens the PSUM accumulation), then
            # off-diagonals, then the narrow diagonals
            kcs = [4 * J] + list(range(4 * J)) + [4 * J + i for i in range(1, 4)]

            def issue_sc_exp(kc, use_dve, use_sc2=False, J=J, hp=hp):
                off = max(0, 128 * (kc - 4 * J))
                w = 512 - off
                if use_sc2:
                    sc = bps2.tile([128, 1024], F32, name="sc2", tag="sc2", bufs=1)
                else:
                    sc = bps.tile([128, 1024], F32, name="sc", tag="sc", bufs=2)
                for hh in range(2):
                    nc.tensor.matmul(
                        sc[:, hh * 512:hh * 512 + w],
                        kT[hp][hh * 64:(hh + 1) * 64, kc * 128:(kc + 1) * 128],
                        qT[hp][hh * 64:(hh + 1) * 64, J * 512 + off:(J + 1) * 512],
                        start=True, stop=True,
                        tile_position=(hh * 64, 0),
                    )
                ex = sbuf.tile([128, 1024], ATT, name="ex", tag="ex", bufs=8)
                if use_dve == "split":
                    # both engines on one chunk (one head each): halves the
                    # exp latency in the sc-bank reuse chain, which is what
                    # paces the big J blocks once projection filler runs out
                    nc.scalar.activation(
                        ex[:, 0:w], sc[:, 0:w], AF.Exp, scale=8.0, bias=lnk[:]
                    )
                    nc.vector._custom_dve(
                        exp8m, out=ex[:, 512:512 + w], in0=sc[:, 512:512 + w],
                        s0=EXP_A2, s1=EXP_A1, imm2=EXP_A0,
                    )
                else:
                    exs = ex[:].rearrange("p (h c) -> p h c", c=512)[:, :, 0:w]
                    scs = sc[:].rearrange("p (h c) -> p h c", c=512)[:, :, 0:w]
                    if use_dve:
                        nc.vector._custom_dve(
                            exp8m, out=exs, in0=scs,
                            s0=EXP_A2, s1=EXP_A1, imm2=EXP_A0,
                        )
                    else:
                        nc.scalar.activation(exs, scs, AF.Exp, scale=8.0, bias=lnk[:])
                if off or kc == 4 * J:
                    # causal mask on the diagonal 128 queries of both heads in
                    # one gpsimd op (head-axis coefficient 0): keep col >= p
                    mk = ex[:].rearrange("p (h c) -> p h c", c=512)[:, :, 0:128]
                    nc.gpsimd.affine_select(
                        out=mk, in_=mk, compare_op=OP.is_ge, fill=0.0,
                        base=0, pattern=[[0, 2], [1, 128]],
                        channel_multiplier=-1,
                    )
                return ex

            def issue_av(kc, ex, first, last, J=J, hp=hp, at=at):
                off = max(0, 128 * (kc - 4 * J))
                w = 512 - off
                for hh in range(2):
                    h = 2 * hp + hh
                    nc.tensor.matmul(
                        at[hh][:, off:512],
                        vt[kc][:, h * 128:(h + 1) * 128],
                        ex[:, hh * 512:hh * 512 + w],
                        start=first, stop=last,
                    )

            # while the previous block's normalize is outstanding this
            # block's AVs are deferred (an early AV would wait on the at-bank
            # release inside the in-order PE queue and deadlock against the
            # normalize chain); once the multiply is issued AVs drain with
            # one chunk of lookahead
            rti = 1 if n_kc == 4 else 2
            pace_a = max(1, (2 * n_kc) // 8)
            avq = []
            rbs_pending = None
            for ti, kc in enumerate(kcs):
                mode = ti % 8 in (2, 5, 7)
                ex = issue_sc_exp(kc, use_dve=mode, use_sc2=(J == 3 and ti % 3 == 2))
                if pending_norm is not None:
                    if ti == rti:
                        rbs_pending = normalize_recip(*pending_norm)
                    if ti == rti + 2:
                        normalize_mult(*pending_norm, rbs_pending)
                        if pending_norm[0] == 1:  # hp==1: query block J done
                            Jd = pending_norm[1]
                            c_q.extend(
                                (lambda t=t: c_block(t))
                                for t in range(4 * Jd, 4 * Jd + 4)
                            )
                        pending_norm = None
                        rbs_pending = None
                avq.append((kc, ex, ti))
                if pending_norm is None:
                    while len(avq) > 1:
                        k0, e0, t0 = avq.pop(0)
                        issue_av(k0, e0, first=(t0 == 0), last=False)
                if a_q and ti % pace_a == 0:
                    a_q.pop(0)()
                if c_q and ti % 3 == 0:
                    c_q.pop(0)()
            while len(avq) > 1:
                k0, e0, t0 = avq.pop(0)
                issue_av(k0, e0, first=(t0 == 0), last=False)
            k0, e0, t0 = avq.pop(0)
            issue_av(k0, e0, first=(t0 == 0), last=True)
            while a_q and hp == 1:
                a_q.pop(0)()
            pending_norm = (hp, J, at, issue_srows(at))

        # tail: the last block's (3,1) normalize is still outstanding, but
        # attT[0] for J=3 is already normalized — run the final C blocks'
        # hp=0 accumulation halves under the normalize chain
        while c_q:
            c_q.pop(0)()
        tails = []
        for t in (12, 13, 14):
            if t == 14:
                op = bps2.tile([128, 1024], F32, name="cop2", tag="sc2", bufs=1)
            else:
                op = bps.tile([128, 1024], F32, name="cop", tag="sc", bufs=2)
            for nn in range(2):
                nc.tensor.matmul(
                    op[:, nn * 512:(nn + 1) * 512],
                    attT[0][:, t * 128:(t + 1) * 128],
                    wo_r[0][:, nn * 512:(nn + 1) * 512],
                    start=True, stop=False,
                )
            tails.append((t, op))
        rbs_pending = normalize_recip(*pending_norm)
        normalize_mult(*pending_norm, rbs_pending)
        for t, op in tails:
            for nn in range(2):
                nc.tensor.matmul(
                    op[:, nn * 512:(nn + 1) * 512],
                    attT[1][:, t * 128:(t + 1) * 128],
                    wo_r[1][:, nn * 512:(nn + 1) * 512],
                    start=False, stop=True,
                )
            ot = sbuf.tile([128, D], BF16, name="ot", tag="ot", bufs=3)
            if t % 2 == 0:
                nc.scalar.copy(ot[:], op[:])
            else:
                nc.vector.tensor_copy(ot[:], op[:])
            nc.sync.dma_start(O[t * 128:(t + 1) * 128, :], ot[:])
        c_block(15)

    nc.compile()
    _NC_CACHE[key] = nc
    return nc


def _prep_core_inputs(H, key_padding_mask, Wq, bq, Wk, bk, Wv, bv, Wo, bo):
    keep = 1.0 - np.asarray(key_padding_mask, dtype=np.float32)  # [B, T]
    in_maps = []
    for c in range(8):
        b, g = divmod(c, 4)
        sl = slice(g * GD, (g + 1) * GD)
        WvT = Wv[sl].T  # [D, GD]
        WvS = np.zeros((D, GV), dtype=np.float32)
        bvS = np.zeros((1, GV), dtype=np.float32)
        for h in range(HPC):
            WvS[:, h * 65:h * 65 + 64] = WvT[:, h * 64:(h + 1) * 64]
            bvS[0, h * 65:h * 65 + 64] = bv[sl][h * 64:(h + 1) * 64]
            bvS[0, h * 65 + 64] = 1.0
        bf = ml_dtypes.bfloat16
        # q is scaled by 1/64 so PSUM scores arrive as u = 0.125*r/8, the
        # operating range of the EXP8M cubic; the ACT path applies scale=8.
        in_maps.append({
            "HT": np.ascontiguousarray(H[b].T).astype(bf),
            "WqT": np.ascontiguousarray(Wq[sl].T / 64.0).astype(bf),
            "WkT": np.ascontiguousarray(Wk[sl].T).astype(bf),
            "WvS": WvS.astype(bf),
            "WoS": np.ascontiguousarray(Wo[:, sl].T),
            "bq": np.ascontiguousarray(bq[sl][None, :] / 64.0).astype(bf),
            "bk": np.ascontiguousarray(bk[sl][None, :]).astype(bf),
            "bvS": bvS.astype(bf),
            "kpm": np.ascontiguousarray(keep[b].reshape(NT, 128).T),
        })
    return in_maps


def kernel(H, key_padding_mask, Wq, bq, Wk, bk, Wv, bv, Wo, bo, _run_kwargs=None):
    H = np.asarray(H, dtype=np.float32)
    Wq = np.asarray(Wq, dtype=np.float32)
    Wk = np.asarray(Wk, dtype=np.float32)
    Wv = np.asarray(Wv, dtype=np.float32)
    Wo = np.asarray(Wo, dtype=np.float32)
    bq = np.asarray(bq, dtype=np.float32)
    bk = np.asarray(bk, dtype=np.float32)
    bv = np.asarray(bv, dtype=np.float32)
    bo = np.asarray(bo, dtype=np.float32)

    # all-zero q/k biases contribute exactly nothing, and an all-False key
    # padding mask keeps every key: skip the corresponding instructions
    nc = build(
        skip_qk_bias=not (bq.any() or bk.any()),
        skip_kpm=not np.asarray(key_padding_mask).any(),
    )
    in_maps = _prep_core_inputs(H, key_padding_mask, Wq, bq, Wk, bk, Wv, bv, Wo, bo)
    res = bass_utils.run_bass_kernel_spmd(
        nc, in_maps, core_ids=list(range(8)), **(_run_kwargs or {})
    )
    out = np.zeros((B, T, D), dtype=np.float32)
    for c in range(8):
        out[c // 4] += np.asarray(res.results[c]["O"], dtype=np.float32)
    out += bo
    if _run_kwargs:
        kernel.last_result = res
    return out


# revision 66
# speedup vs baseline: 1.1976x; 1.0145x over previous
"""Multi-head self-attention (B=2, T=2048, D=1024, 16 heads) on 8 TRN2 cores.

Sharding: core c = (b, g) with b = c // 4 (batch), g = c % 4 (head group of 4).
Each core computes q/k/v projections for its 4 heads, causal softmax
attention, and a partial output projection (its 256 columns of the
concat-head dim against Wo). Host sums the 4 partials per batch and adds bo.

Single merged pipeline, ordered so the PE never idles (keeps the HAM clock
gate warm): projection "granules" (one 512-token key block: kT, vt, qT) are
interleaved chunk-by-chunk with the attention stream of the previous key
block and with output-projection blocks of the block before that.

  granule n:  kT/qT [256, 512-slice] (transposed projections, head pairs
              stacked on partitions) and vt [512, 260] natural (per head 64
              value cols + a ones col that makes the AV matmul emit softmax
              denominators).
  B(J, hp):   per key-chunk kc: scoresT [128, 2x512] for both heads via
              tile_position row packing; exp on the scalar ACT (Exp LUT,
              scale=8, bias=ln K) or the vector engine (custom EXP8M op:
              monic cubic + 3 squarings, K-scaled) -- both emit
              K*exp(0.125 r) so K cancels in the softmax; causal masks via
              gpsimd affine_select; AV accumulates [v|1].T @ ex into
              at [65, 512] (row 64 = denominators). Normalize: denominators
              staged by scalar copy, reciprocal_approx_fast on DVE, gpsimd
              partition broadcast, DVE multiply into attT -- staged across
              the next block so no in-order engine queue ever stalls.
  C(t):       O[t-block] = attT.T @ WoS accumulated over head pairs in PSUM,
              copied to bf16 and DMA'd out; host converts/sums partials.
"""

import ml_dtypes
import numpy as np

import concourse.bass as bass
import concourse.tile as tile
from concourse import bacc, mybir
from concourse import bass_utils
from concourse import dve_ops
from concourse.dve_ops import DveOp
from concourse.dve_spec import Spec, Src0, C0, C1, C2, sq, lower as dve_lower
from concourse.dve_uop import DveOpSpec
from contextlib import ExitStack

F32 = mybir.dt.float32
F32R = mybir.dt.float32r
BF16 = mybir.dt.bfloat16
ATT = BF16  # dtype for attention-phase matmul operands
AF = mybir.ActivationFunctionType
OP = mybir.AluOpType

B, T, D = 2, 2048, 1024
NH, DH = 16, 64
HPC = 4            # heads per core
GD = HPC * DH      # 256, group dim
GV = HPC * (DH + 1)  # 260, v tile width
NKD = D // 128     # 8 K-chunks for projections
NT = T // 128      # 16 token chunks
NJ = T // 512      # 4 query blocks

_NC_CACHE = {}

# exp via monic cubic + 3 squarings on the vector engine:
#   p(u) = ((u + A2) u + A1) u + A0;  p(u)^8 ~= EXPK * exp(8u)  on |u| <= 0.47
# (max rel err 2.2e-4 in fp32). Scores arrive in PSUM pre-scaled to u = r/64
# (0.125/8 folded into Wq host-side); the ACT path uses scale=8, bias=ln(EXPK)
# so both engines emit EXPK * exp(0.125 r) and EXPK cancels in the softmax.
EXP_A2 = 3.06702906
EXP_A1 = 6.02255865
EXP_A0 = 6.01835402
EXP_LNK = 14.36056232


def _register_exp8m():
    if "EXP8M" in dve_ops._SUB_OPCODE_FOR_NAME:
        return next(o for o in dve_ops.OPS if o.name == "EXP8M")
    _p = ((Src0 + C0) * Src0 + C1) * Src0 + C2
    _body = sq(sq(sq(_p)))

    def _ref(in0, in1, s0, s1, imm2):
        p = (((in0 + s0) * in0 + s1) * in0 + imm2).astype(np.float32)
        return ((p * p) ** 2) ** 2

    spec = Spec(body=_body, reference=_ref)
    ver = "v3"
    sha = DveOpSpec(name="EXP8M", opcode=1, uops=dve_lower(spec, ver=ver),
                    rd1_en=False).sha(ver)
    op = DveOp("EXP8M", spec, subdim=False, uops_sha={ver: sha})
    dve_ops.OPS.append(op)
    dve_ops.CUSTOM_DVE_SPECS[op.name] = op.spec
    dve_ops._SUB_OPCODE_FOR_NAME[op.name] = (
        dve_ops._CUSTOM_DVE_ROW_BASE + len(dve_ops.OPS) - 1
    )
    return op


def build(skip_qk_bias=False, skip_kpm=False):
    key = ("nc", skip_qk_bias, skip_kpm)
    if key in _NC_CACHE:
        return _NC_CACHE[key]
    exp8m = _register_exp8m()
    nc = bacc.Bacc("TRN2", target_bir_lowering=False, debug=False, num_devices=8)

    HT = nc.dram_tensor("HT", [D, T], BF16, kind="ExternalInput").ap()
    WqT = nc.dram_tensor("WqT", [D, GD], BF16, kind="ExternalInput").ap()
    WkT = nc.dram_tensor("WkT", [D, GD], BF16, kind="ExternalInput").ap()
    WvS = nc.dram_tensor("WvS", [D, GV], BF16, kind="ExternalInput").ap()
    WoS = nc.dram_tensor("WoS", [GD, D], F32R, kind="ExternalInput").ap()
    bq = nc.dram_tensor("bq", [1, GD], BF16, kind="ExternalInput").ap()
    bk = nc.dram_tensor("bk", [1, GD], BF16, kind="ExternalInput").ap()
    bvS = nc.dram_tensor("bvS", [1, GV], BF16, kind="ExternalInput").ap()
    kpm = nc.dram_tensor("kpm", [128, NT], F32, kind="ExternalInput").ap()
    O = nc.dram_tensor("O", [T, D], BF16, kind="ExternalOutput").ap()

    ENGS = [nc.sync, nc.scalar, nc.gpsimd]

    with tile.TileContext(nc) as tc, ExitStack() as octx:
        cpool = octx.enter_context(tc.tile_pool(name="const", bufs=1))
        keep = octx.enter_context(tc.tile_pool(name="keep", bufs=1))
        sbuf = octx.enter_context(tc.tile_pool(name="work", bufs=1))
        bps = octx.enter_context(tc.tile_pool(name="bps", bufs=1, space="PSUM"))
        aps_ctx = ExitStack()
        aps = aps_ctx.enter_context(tc.tile_pool(name="aps", bufs=1, space="PSUM"))
        bps2 = None  # opened once projection PSUM frees (start of J=3)

        # ---- constants ----
        ones_f = cpool.tile([1, 512], F32, name="ones_f", tag="ones_f")
        nc.vector.memset(ones_f[:], 1.0)
        ones_r = cpool.tile([1, 512], BF16, name="ones_r", tag="ones_r")
        nc.vector.tensor_copy(ones_r[:], ones_f[:])

        bq_r = cpool.tile([1, GD], BF16, name="bq_r", tag="bq_r")
        bk_r = cpool.tile([1, GD], BF16, name="bk_r", tag="bk_r")
        bv_r = cpool.tile([1, GV], BF16, name="bv_r", tag="bv_r")
        kpm_sb = cpool.tile([128, NT], F32, name="kpm_sb", tag="kpm_sb")

        lnk = cpool.tile([128, 1], F32, name="lnk", tag="lnk")
        nc.vector.memset(lnk[:], EXP_LNK)

        # ---- long-lived activations ----
        qT = [keep.tile([128, T], ATT, name=f"qT{m}", tag=f"qT{m}") for m in range(2)]
        kT = [keep.tile([128, T], ATT, name=f"kT{m}", tag=f"kT{m}") for m in range(2)]
        vt = [keep.tile([128, 512], ATT, name=f"vt{t}", tag=f"vt{t}") for t in range(NT)]
        attT = [keep.tile([128, T], F32R, name=f"attT{m}", tag=f"attT{m}") for m in range(2)]
        wo_r = [keep.tile([128, D], F32R, name=f"wo{i}", tag=f"wo{i}") for i in range(2)]
        ht_r = [keep.tile([128, T], BF16, name=f"ht{k}", tag=f"ht{k}") for k in range(NKD)]
        wq_r = sbuf.tile([128, NKD * GD], BF16, name="wq_r", tag="wq_r")
        wk_r = sbuf.tile([128, NKD * GD], BF16, name="wk_r", tag="wk_r")
        wv_r = sbuf.tile([128, NKD * GV], BF16, name="wv_r", tag="wv_r")

        for t in range(NT):
            nc.vector.memset(vt[t][:], 0.0)

        # ---- input DMA, ordered for earliest granule-0 start: the first
        # issue on each engine queue is data the very first matmuls need ----
        for k in range(NKD):
            nc.sync.dma_start(wk_r[:, k * GD:(k + 1) * GD], WkT[k * 128:(k + 1) * 128, :])
            nc.scalar.dma_start(ht_r[k][:, 0:512], HT[k * 128:(k + 1) * 128, 0:512])
            nc.gpsimd.dma_start(wv_r[:, k * GV:(k + 1) * GV], WvS[k * 128:(k + 1) * 128, :])
        if not skip_qk_bias:
            nc.sync.dma_start(bk_r[:], bk[:])
        nc.gpsimd.dma_start(bv_r[:], bvS[:])
        if not skip_kpm:
            nc.gpsimd.dma_start(kpm_sb[:], kpm[:])
        if not skip_qk_bias:
            nc.sync.dma_start(bq_r[:], bq[:])
        for k in range(NKD):
            nc.sync.dma_start(wq_r[:, k * GD:(k + 1) * GD], WqT[k * 128:(k + 1) * 128, :])
            nc.scalar.dma_start(ht_r[k][:, 512:1024], HT[k * 128:(k + 1) * 128, 512:1024])
            nc.gpsimd.dma_start(ht_r[k][:, 1024:1536], HT[k * 128:(k + 1) * 128, 1024:1536])
        for i in range(2):
            nc.gpsimd.dma_start(wo_r[i][:], WoS[i * 128:(i + 1) * 128, :])
        for k in range(NKD):
            ENGS[k % 3].dma_start(
                ht_r[k][:, 1536:2048], HT[k * 128:(k + 1) * 128, 1536:2048]
            )

        # ---------- projection granule work (phase A) ----------
        def proj_group(w_r, dest, brow, m, n):
            ps = aps.tile([128, 512], F32, name="ps", tag="aps", bufs=2)
            for k in range(NKD):
                nc.tensor.matmul(
                    ps[:],
                    w_r[:, k * GD + m * 128: k * GD + m * 128 + 128],
                    ht_r[k][:, n * 512:(n + 1) * 512],
                    start=(k == 0),
                    stop=(skip_qk_bias and k == NKD - 1),
                )
            if not skip_qk_bias:
                nc.tensor.matmul(
                    ps[:], brow[:, m * 128:(m + 1) * 128], ones_r[:],
                    start=False, stop=True,
                )
            nc.vector.tensor_copy(dest[m][:, n * 512:(n + 1) * 512], ps[:])

        def vt_group(t):
            vp = aps.tile([128, 512], F32, name="vp", tag="aps", bufs=2)
            for k in range(NKD):
                nc.tensor.matmul(
                    vp[:, 0:GV],
                    ht_r[k][:, t * 128:(t + 1) * 128],
                    wv_r[:, k * GV:(k + 1) * GV],
                    start=(k == 0), stop=False,
                )
            nc.tensor.matmul(
                vp[:, 0:GV], ones_r[:, 0:128], bv_r[:], start=False, stop=True
            )
            nc.vector.tensor_copy(
                vt[t][:].rearrange("p (h c) -> p h c", c=128)[:, :, 0:65],
                vp[:, 0:GV].rearrange("p (h c) -> p h c", c=65),
            )
            if not skip_kpm:
                nc.vector.tensor_scalar_mul(vt[t][:], vt[t][:], kpm_sb[:, t:t + 1])

        def a_granule_groups(n):
            gs = []
            for m in range(2):
                gs.append(lambda m=m: proj_group(wk_r, kT, bk_r, m, n))
            for tt in range(4 * n, 4 * n + 4):
                gs.append(lambda tt=tt: vt_group(tt))
            for m in range(2):
                gs.append(lambda m=m: proj_group(wq_r, qT, bq_r, m, n))
            return gs

        # ---------- output projection blocks (phase C) ----------
        c_flip = [0]

        def c_block(t):
            op = bps.tile([128, 1024], F32, name="cop", tag="sc", bufs=2)
            for nn in range(2):
                for hp2 in range(2):
                    nc.tensor.matmul(
                        op[:, nn * 512:(nn + 1) * 512],
                        attT[hp2][:, t * 128:(t + 1) * 128],
                        wo_r[hp2][:, nn * 512:(nn + 1) * 512],
                        start=(hp2 == 0), stop=(hp2 == 1),
                    )
            ot = sbuf.tile([128, D], BF16, name="ot", tag="ot", bufs=3)
            if c_flip[0] % 2 == 0:
                nc.scalar.copy(ot[:], op[:])
            else:
                nc.vector.tensor_copy(ot[:], op[:])
            c_flip[0] += 1
            nc.sync.dma_start(O[t * 128:(t + 1) * 128, :], ot[:])

        # ---------- attention (phase B) with normalize staging ----------
        def issue_srows(at):
            # stage the PSUM denominator rows (partition 64) to SBUF on the
            # scalar engine (DMA cannot read PSUM); issued at block end so
            # the next block's reciprocal finds them ready
            srows = []
            for hh in range(2):
                srow = sbuf.tile([1, 512], F32, name="srow", tag="srow", bufs=4)
                nc.scalar.copy(srow[:], at[hh][64:65, :])
                srows.append(srow)
            return srows

        def normalize_recip(hp, J, at, srows):
            rbs = []
            for hh in range(2):
                rc = sbuf.tile([1, 512], F32, name="rc", tag="rc", bufs=4)
                nc.vector.reciprocal_approx_fast(out=rc[:], in_=srows[hh][:])
                rb = sbuf.tile([64, 512], F32, name="rb", tag="rb", bufs=4)
                nc.gpsimd.partition_broadcast(rb[:], rc[:])
                rbs.append(rb)
            return rbs

        def normalize_mult(hp, J, at, srows, rbs):
            for hh in range(2):
                nc.vector.tensor_tensor(
                    attT[hp][hh * 64:(hh + 1) * 64, J * 512:(J + 1) * 512],
                    at[hh][0:64, :],
                    rbs[hh][:],
                    op=OP.mult,
                )

        # granule 0 runs alone up front
        for g in a_granule_groups(0):
            g()

        a_q = []
        c_q = []
        pending_norm = None
        order = [(0, 0), (0, 1), (1, 0), (1, 1), (2, 0), (2, 1), (3, 0), (3, 1)]
        for bi, (J, hp) in enumerate(order):
            if hp == 0 and J < 3:
                a_q.extend(a_granule_groups(J + 1))
            if bi == 6:
                # all projection work has drained: release its 2 PSUM banks
                # and open a third scores slot, deepening the exp->scores
                # bank-reuse chain from 2 to 3 for the chain-paced J=3 blocks
                assert not a_q
                aps_ctx.close()
                bps2 = octx.enter_context(
                    tc.tile_pool(name="bps2", bufs=1, space="PSUM")
                )
            n_kc = 4 * J + 4
            at = [
                bps.tile([128, 512], F32, name=f"at{hh}", tag="av", bufs=2)
                for hh in range(2)
            ]
            # diagonal first (full width, opens the PSUM accumulation), then
            # off-diagonals, then the narrow diagonals
            kcs = [4 * J] + list(range(4 * J)) + [4 * J + i for i in range(1, 4)]

            def issue_sc_exp(kc, use_dve, use_sc2=False, J=J, hp=hp):
                off = max(0, 128 * (kc - 4 * J))
                w = 512 - off
                if use_sc2:
                    sc = bps2.tile([128, 1024], F32, name="sc2", tag="sc2", bufs=1)
                else:
                    sc = bps.tile([128, 1024], F32, name="sc", tag="sc", bufs=2)
                for hh in range(2):
                    nc.tensor.matmul(
                        sc[:, hh * 512:hh * 512 + w],
                        kT[hp][hh * 64:(hh + 1) * 64, kc * 128:(kc + 1) * 128],
                        qT[hp][hh * 64:(hh + 1) * 64, J * 512 + off:(J + 1) * 512],
                        start=True, stop=True,
                        tile_position=(hh * 64, 0),
                    )
                ex = sbuf.tile([128, 1024], ATT, name="ex", tag="ex", bufs=8)
                if use_dve == "split":
                    # both engines on one chunk (one head each): halves the
                    # exp latency in the sc-bank reuse chain, which is what
                    # paces the big J blocks once projection filler runs out
                    nc.scalar.activation(
                        ex[:, 0:w], sc[:, 0:w], AF.Exp, scale=8.0, bias=lnk[:]
                    )
                    nc.vector._custom_dve(
                        exp8m, out=ex[:, 512:512 + w], in0=sc[:, 512:512 + w],
                        s0=EXP_A2, s1=EXP_A1, imm2=EXP_A0,
                    )
                else:
                    exs = ex[:].rearrange("p (h c) -> p h c", c=512)[:, :, 0:w]
                    scs = sc[:].rearrange("p (h c) -> p h c", c=512)[:, :, 0:w]
                    if use_dve:
                        nc.vector._custom_dve(
                            exp8m, out=exs, in0=scs,
                            s0=EXP_A2, s1=EXP_A1, imm2=EXP_A0,
                        )
                    else:
                        nc.scalar.activation(exs, scs, AF.Exp, scale=8.0, bias=lnk[:])
                if off or kc == 4 * J:
                    # causal mask on the diagonal 128 queries of both heads in
                    # one gpsimd op (head-axis coefficient 0): keep col >= p
                    mk = ex[:].rearrange("p (h c) -> p h c", c=512)[:, :, 0:128]
                    nc.gpsimd.affine_select(
                        out=mk, in_=mk, compare_op=OP.is_ge, fill=0.0,
                        base=0, pattern=[[0, 2], [1, 128]],
                        channel_multiplier=-1,
                    )
                return ex

            def issue_av(kc, ex, first, last, J=J, hp=hp, at=at):
                off = max(0, 128 * (kc - 4 * J))
                w = 512 - off
                for hh in range(2):
                    h = 2 * hp + hh
                    nc.tensor.matmul(
                        at[hh][:, off:512],
                        vt[kc][:, h * 128:(h + 1) * 128],
                        ex[:, hh * 512:hh * 512 + w],
                        start=first, stop=last,
                    )

            # while the previous block's normalize is outstanding this
            # block's AVs are deferred (an early AV would wait on the at-bank
            # release inside the in-order PE queue and deadlock against the
            # normalize chain); once the multiply is issued AVs drain with
            # one chunk of lookahead
            rti = 1 if n_kc == 4 else 2
            pace_a = max(1, (2 * n_kc) // 8)
            avq = []
            rbs_pending = None
            for ti, kc in enumerate(kcs):
                mode = ti % 8 in (2, 5, 7)
                ex = issue_sc_exp(kc, use_dve=mode, use_sc2=(J == 3 and ti % 3 == 2))
                if pending_norm is not None:
                    if ti == rti:
                        rbs_pending = normalize_recip(*pending_norm)
                    if ti == rti + 2:
                        normalize_mult(*pending_norm, rbs_pending)
                        if pending_norm[0] == 1:  # hp==1: query block J done
                            Jd = pending_norm[1]
                            c_q.extend(
                                (lambda t=t: c_block(t))
                                for t in range(4 * Jd, 4 * Jd + 4)
                            )
                        pending_norm = None
                        rbs_pending = None
                avq.append((kc, ex, ti))
                if pending_norm is None:
                    while len(avq) > 1:
                        k0, e0, t0 = avq.pop(0)
                        issue_av(k0, e0, first=(t0 == 0), last=False)
                if a_q and ti % pace_a == 0:
                    a_q.pop(0)()
                if c_q and ti % 3 == 0:
                    c_q.pop(0)()
            while len(avq) > 1:
                k0, e0, t0 = avq.pop(0)
                issue_av(k0, e0, first=(t0 == 0), last=False)
            k0, e0, t0 = avq.pop(0)
            issue_av(k0, e0, first=(t0 == 0), last=True)
            while a_q and hp == 1:
                a_q.pop(0)()
            pending_norm = (hp, J, at, issue_srows(at))

        # tail: the last block's (3,1) normalize is still outstanding, but
        # attT[0] for J=3 is already normalized — run the final C blocks'
        # hp=0 accumulation halves under the normalize chain
        while c_q:
            c_q.pop(0)()
        tails = []
        for t in (12, 13, 14):
            if t == 14:
                op = bps2.tile([128, 1024], F32, name="cop2", tag="sc2", bufs=1)
            else:
                op = bps.tile([128, 1024], F32, name="cop", tag="sc", bufs=2)
            for nn in range(2):
                nc.tensor.matmul(
                    op[:, nn * 512:(nn + 1) * 512],
                    attT[0][:, t * 128:(t + 1) * 128],
                    wo_r[0][:, nn * 512:(nn + 1) * 512],
                    start=True, stop=False,
                )
            tails.append((t, op))
        rbs_pending = normalize_recip(*pending_norm)
        normalize_mult(*pending_norm, rbs_pending)
        for t, op in tails:
            for nn in range(2):
                nc.tensor.matmul(
                    op[:, nn * 512:(nn + 1) * 512],
                    attT[1][:, t * 128:(t + 1) * 128],
                    wo_r[1][:, nn * 512:(nn + 1) * 512],
                    start=False, stop=True,
                )
            ot = sbuf.tile([128, D], BF16, name="ot", tag="ot", bufs=3)
            if t % 2 == 0:
                nc.scalar.copy(ot[:], op[:])
            else:
                nc.vector.tensor_copy(ot[:], op[:])
            nc.sync.dma_start(O[t * 128:(t + 1) * 128, :], ot[:])
        c_block(15)

    nc.compile()
    _NC_CACHE[key] = nc
    return nc


def _prep_core_inputs(H, key_padding_mask, Wq, bq, Wk, bk, Wv, bv, Wo, bo):
    keep = 1.0 - np.asarray(key_padding_mask, dtype=np.float32)  # [B, T]
    in_maps = []
    for c in range(8):
        b, g = divmod(c, 4)
        sl = slice(g * GD, (g + 1) * GD)
        WvT = Wv[sl].T  # [D, GD]
        WvS = np.zeros((D, GV), dtype=np.float32)
        bvS = np.zeros((1, GV), dtype=np.float32)
        for h in range(HPC):
            WvS[:, h * 65:h * 65 + 64] = WvT[:, h * 64:(h + 1) * 64]
            bvS[0, h * 65:h * 65 + 64] = bv[sl][h * 64:(h + 1) * 64]
            bvS[0, h * 65 + 64] = 1.0
        bf = ml_dtypes.bfloat16
        # q is scaled by 1/64 so PSUM scores arrive as u = 0.125*r/8, the
        # operating range of the EXP8M cubic; the ACT path applies scale=8.
        in_maps.append({
            "HT": np.ascontiguousarray(H[b].T).astype(bf),
            "WqT": np.ascontiguousarray(Wq[sl].T / 64.0).astype(bf),
            "WkT": np.ascontiguousarray(Wk[sl].T).astype(bf),
            "WvS": WvS.astype(bf),
            "WoS": np.ascontiguousarray(Wo[:, sl].T),
            "bq": np.ascontiguousarray(bq[sl][None, :] / 64.0).astype(bf),
            "bk": np.ascontiguousarray(bk[sl][None, :]).astype(bf),
            "bvS": bvS.astype(bf),
            "kpm": np.ascontiguousarray(keep[b].reshape(NT, 128).T),
        })
    return in_maps


def kernel(H, key_padding_mask, Wq, bq, Wk, bk, Wv, bv, Wo, bo, _run_kwargs=None):
    H = np.asarray(H, dtype=np.float32)
    Wq = np.asarray(Wq, dtype=np.float32)
    Wk = np.asarray(Wk, dtype=np.float32)
    Wv = np.asarray(Wv, dtype=np.float32)
    Wo = np.asarray(Wo, dtype=np.float32)
    bq = np.asarray(bq, dtype=np.float32)
    bk = np.asarray(bk, dtype=np.float32)
    bv = np.asarray(bv, dtype=np.float32)
    bo = np.asarray(bo, dtype=np.float32)

    # all-zero q/k biases contribute exactly nothing, and an all-False key
    # padding mask keeps every key: skip the corresponding instructions
    nc = build(
        skip_qk_bias=not (bq.any() or bk.any()),
        skip_kpm=not np.asarray(key_padding_mask).any(),
    )
    in_maps = _prep_core_inputs(H, key_padding_mask, Wq, bq, Wk, bk, Wv, bv, Wo, bo)
    res = bass_utils.run_bass_kernel_spmd(
        nc, in_maps, core_ids=list(range(8)), **(_run_kwargs or {})
    )
    out = np.zeros((B, T, D), dtype=np.float32)
    for c in range(8):
        out[c // 4] += np.asarray(res.results[c]["O"], dtype=np.float32)
    out += bo
    if _run_kwargs:
        kernel.last_result = res
    return out
